# revision 18
# baseline (speedup 1.0000x reference)
"""Contrastive-loss kernel for Trainium2 (8 NeuronCores, Bass/Tile).

Math (reference):
    W = wsi[:, 0, :], O = omic[:, 0, :]                      # [N, D]
    S = (W @ O.T) / max(|W_i||O_j|, eps)                     # [N, N] cosine sims
    d = diag(S)
    L = where(eye, 1 - S, relu(M - S + d[:, None]))
    out = mean(L)

Device identity (no diagonal masking of the [N, N] block needed):
    sum(L) = sum_{i,j} relu(hb_i - S_ij) + sum_i [(1 - d_i) - relu(hb_i - S_ii)]
    with hb_i = M + d_i. Since hb_i - S_ii ~= M > 0, the per-row correction is
    (1 - M - 2 d_i) + S_ii: the device ships row-sums of relu(hb - S) and the
    diagonal entries S_ii; the analytic part is added on the host.

Distribution: data-parallel over W rows. Each core c gets its 512 W rows
(pre-normalized, fp8-e4m3, DoubleRow-packed) plus the full normalized O,
column-rotated by 512*c so the diagonal block always lands in j-chunk 0
(keeps the SPMD program core-independent). The exact diagonal bias hb is
computed on the host in f64 and shipped, which removes the row-correlated
part of the fp8 quantization error.

Schedule: for each i-tile t, the 8 j-chunk PSUM banks accumulate together
with dd outer / jc inner, so each DoubleRow LDWEIGHTS (213 ns for 256
columns) is reused by 8 matmuls (107 ns each) instead of reloading per
matmul. The hinge is split: ScalarE applies relu(hb - S) for j-chunks 0-5
(bf16 out), VectorE computes min(S - hb, 0) for j-chunks 6-7; row-sums are
consolidated into one wide DVE reduce per group, and one ones-matmul
collapses partitions so the output DMA is a single partition line.
"""

import numpy as np
import ml_dtypes

N = 4096
D = 1024
NCORES = 8
ROWS = N // NCORES  # 512 W rows per core
P = 128             # SBUF partitions
NJ = 512            # moving free dim per matmul (one PSUM bank of fp32)
TI = ROWS // P      # 4 i-tiles per core
ND2 = D // 256      # 4 DoubleRow contraction chunks (256 deep each)
NJC = N // NJ       # 8 j-chunks
NJC_ACT = 6         # j-chunks hinged on ScalarE (rest on VectorE)
MARGIN = 0.1
N_WARMUP = 18       # wide bf16 PE-warmup matmuls issued while DMAs stream
NCOL = TI * 3       # per t: [ACT rowsum, -(DVE rowsum), diag S_ii sum]

_cache = {}


def _build():
    from contextlib import ExitStack
    import concourse.bacc as bacc
    import concourse.tile as tile
    import concourse.mybir as mybir

    f32 = mybir.dt.float32
    bf16 = mybir.dt.bfloat16
    fp8 = mybir.dt.float8e4

    nc = bacc.Bacc("TRN2", target_bir_lowering=False, debug=False,
                   num_devices=NCORES)
    wt_d = nc.dram_tensor("wt", [P, TI * ND2, 2, P], fp8,
                          kind="ExternalInput").ap()
    ot_d = nc.dram_tensor("ot", [P, ND2, NJC, 2, NJ], fp8,
                          kind="ExternalInput").ap()
    id_d = nc.dram_tensor("id", [P, P], f32, kind="ExternalInput").ap()
    hb_d = nc.dram_tensor("hb", [P, TI], f32, kind="ExternalInput").ap()
    out_d = nc.dram_tensor("out", [1, NCOL], f32, kind="ExternalOutput").ap()

    with tile.TileContext(nc) as tc, ExitStack() as ctx:
        const = ctx.enter_context(tc.tile_pool(name="const", bufs=1))
        pp = ctx.enter_context(tc.tile_pool(name="pp", bufs=8, space="PSUM"))
        scrp = ctx.enter_context(tc.tile_pool(name="scr", bufs=2))
        smallp = ctx.enter_context(tc.tile_pool(name="small", bufs=2))

        # DMA order puts the first sweep's operands (t=0 weights + the dd=0
        # slab across all j-chunks) at the head of the HWDGE queue.
        wt_sb = const.tile([P, TI * ND2, 2, P], fp8, tag="wt")
        nc.sync.dma_start(out=wt_sb[:, 0:ND2, :, :], in_=wt_d[:, 0:ND2, :, :])
        ot_sb = const.tile([P, ND2, NJC, 2, NJ], fp8, tag="ot")
        nc.sync.dma_start(out=ot_sb[:, 0, :, :, :], in_=ot_d[:, 0, :, :, :])
        nc.sync.dma_start(out=wt_sb[:, ND2:, :, :], in_=wt_d[:, ND2:, :, :])
        hb = const.tile([P, TI], f32, tag="hb")
        nc.sync.dma_start(out=hb[:], in_=hb_d[:, :])
        id_sb = const.tile([P, P], f32, tag="id")
        nc.sync.dma_start(out=id_sb[:], in_=id_d[:, :])
        for dd in range(1, ND2):
            nc.sync.dma_start(out=ot_sb[:, dd, :, :, :],
                              in_=ot_d[:, dd, :, :, :])
        ones_sb = const.tile([P, 1], f32, tag="ones")
        nc.vector.memset(ones_sb[:], 1.0)

        # Warm the PE clock (HAM gate releases after ~3.4us of sustained
        # array activity) while the first DMAs stream, so the real matmul
        # stream starts at 2.4 GHz instead of 1.2 GHz.
        warm_w = const.tile([P, 1], bf16, tag="warmw")
        nc.vector.memset(warm_w[:], 0.0)
        warm_rhs = const.tile([P, NJ], bf16, tag="warmrhs")
        nc.vector.memset(warm_rhs[:], 0.0)
        for _ in range(N_WARMUP):
            warm_ps = pp.tile([1, NJ], f32, tag="ps")
            nc.tensor.matmul(warm_ps[:], lhsT=warm_w[:], rhs=warm_rhs[:],
                             start=True, stop=True)

        acc = const.tile([P, NCOL], f32, tag="acc")

        for t in range(TI):
            pss = [pp.tile([P, NJ], f32, tag="ps", name=f"ps_{t}_{j}")
                   for j in range(NJC)]
            for dd in range(ND2):
                for jc in range(NJC):
                    nc.tensor.matmul(
                        pss[jc][:],
                        lhsT=wt_sb[:, t * ND2 + dd, :, :],
                        rhs=ot_sb[:, dd, jc, :, :],
                        start=(dd == 0),
                        stop=(dd == ND2 - 1),
                        perf_mode=mybir.MatmulPerfMode.DoubleRow,
                    )
            # diag column = sum over the identity-masked diag sub-block
            # (tensor_tensor_reduce faults the exec unit on this runtime,
            # so mask + reduce in two DVE ops)
            dprod = scrp.tile([P, P], f32, tag="dprod")
            nc.vector.tensor_mul(dprod[:], pss[0][:, t * P:(t + 1) * P],
                                 id_sb[:])
            nc.vector.tensor_reduce(
                out=acc[:, t * 3 + 2:t * 3 + 3], in_=dprod[:],
                axis=mybir.AxisListType.X, op=mybir.AluOpType.add)
            # hinge: relu(hb - S) on ScalarE for jc < NJC_ACT,
            # min(S - hb, 0) = -relu(hb - S) on VectorE for the rest
            h = scrp.tile([P, NJC_ACT * NJ], bf16, tag="h")
            for jc in range(NJC_ACT):
                nc.scalar.activation(
                    out=h[:, jc * NJ:(jc + 1) * NJ],
                    in_=pss[jc][:],
                    func=mybir.ActivationFunctionType.Relu,
                    bias=hb[:, t:t + 1],
                    scale=-1.0,
                )
            h2 = scrp.tile([P, (NJC - NJC_ACT) * NJ], bf16, tag="h2")
            for k, jc in enumerate(range(NJC_ACT, NJC)):
                nc.vector.tensor_scalar(
                    out=h2[:, k * NJ:(k + 1) * NJ],
                    in0=pss[jc][:],
                    scalar1=hb[:, t:t + 1],
                    scalar2=0.0,
                    op0=mybir.AluOpType.subtract,
                    op1=mybir.AluOpType.min,
                )
            nc.vector.tensor_reduce(
                out=acc[:, t * 3:t * 3 + 1], in_=h[:],
                axis=mybir.AxisListType.X, op=mybir.AluOpType.add)
            nc.vector.tensor_reduce(
                out=acc[:, t * 3 + 1:t * 3 + 2], in_=h2[:],
                axis=mybir.AxisListType.X, op=mybir.AluOpType.add)

        # cross-partition reduce on the PE (ones^T @ acc -> [1, 12]) so the
        # output DMA is one contiguous partition line instead of 128 4-byte
        # descriptors (whose completion receipts dominate the kernel tail)
        tot_ps = pp.tile([1, NCOL], f32, tag="ps")
        nc.tensor.matmul(tot_ps[:], lhsT=ones_sb[:], rhs=acc[:, :],
                         start=True, stop=True)
        total = smallp.tile([1, NCOL], f32, tag="tot")
        nc.vector.tensor_copy(total[:], tot_ps[:])
        nc.sync.dma_start(out=out_d[:, :], in_=total[:])

    nc.compile()
    return nc


def _get_nc():
    if "nc" not in _cache:
        _cache["nc"] = _build()
    return _cache["nc"]


def _prep_inputs(wsi, omic):
    fp8np = ml_dtypes.float8_e4m3
    W = np.asarray(wsi, dtype=np.float32)[:, 0, :].astype(np.float64)
    O = np.asarray(omic, dtype=np.float32)[:, 0, :].astype(np.float64)
    Wn = W / np.maximum(np.linalg.norm(W, axis=1, keepdims=True), 1e-30)
    On = O / np.maximum(np.linalg.norm(O, axis=1, keepdims=True), 1e-30)
    d_exact = np.einsum("nd,nd->n", Wn, On)  # exact cos(w_i, o_i)
    hb_all = (MARGIN + d_exact).astype(np.float32)
    Wn8 = Wn.astype(fp8np)
    On8 = On.astype(fp8np)
    ident = np.eye(P, dtype=np.float32)

    in_maps = []
    for c in range(NCORES):
        Wc = Wn8[c * ROWS:(c + 1) * ROWS]  # [512, 1024]
        # wt[p, t*ND2+dd, r, m] = Wc[t*128+m, dd*256 + r*128 + p]
        wt = np.ascontiguousarray(
            Wc.reshape(TI, P, ND2, 2, P).transpose(4, 0, 2, 3, 1)
            .reshape(P, TI * ND2, 2, P))
        # column rotation: permuted col j' <-> original O row (j' + 512c) % N
        Operm = np.roll(On8, -ROWS * c, axis=0)
        # ot[p, dd, jc, r, n] = Operm[jc*512 + n, dd*256 + r*128 + p]
        ot = np.ascontiguousarray(
            Operm.reshape(NJC, NJ, ND2, 2, P).transpose(4, 2, 0, 3, 1))
        # hb[p, t] = MARGIN + d_exact[c*512 + t*128 + p]
        hbc = np.ascontiguousarray(
            hb_all[c * ROWS:(c + 1) * ROWS].reshape(TI, P).T)
        in_maps.append({"wt": wt, "ot": ot, "id": ident, "hb": hbc})
    return in_maps, d_exact


def kernel(wsi_embeddings, omic_embeddings):
    from concourse.bass_utils import run_bass_kernel_spmd

    nc = _get_nc()
    in_maps, d_exact = _prep_inputs(wsi_embeddings, omic_embeddings)
    res = run_bass_kernel_spmd(nc, in_maps, list(range(NCORES)))
    # device columns per t: [ACT hinge rowsum, -(DVE hinge rowsum), S_ii sum];
    # host adds the analytic per-row correction sum_i (1 - MARGIN - 2 d_i)
    grand = float(np.sum(1.0 - MARGIN - 2.0 * d_exact))
    for c in range(NCORES):
        cols = res.results[c]["out"][0].astype(np.float64)
        for t in range(TI):
            grand += cols[t * 3] - cols[t * 3 + 1] + cols[t * 3 + 2]
    return np.float32(grand / (float(N) * float(N)))
